# revision 33
# baseline (speedup 1.0000x reference)
"""DirectionalContrastiveLoss on 8 TRN2 NeuronCores (Bass/Tile), v7.

Data-parallel over the N=16384 anchor rows (2048 rows/core); the 4000-row
memory bank is replicated (padded to 4096 columns with zero features).

Device algorithm (per core):
- PSUM holds SC*(sim - pos + B0), SC = 184.664 = 2^7/ln2, B0 = 87.99:
  * features as fp8e4m3 DoubleRow matmuls: K=256 contraction in ONE pass
    via the [128, 2, cols] plane AP (planes = the two 128-row K-halves);
    512-col chunks stream at 2 fp8 elem/cycle (~216ns/chunk),
  * label mask as bf16 -SC*1000*onehot(label) x onehot(mem_label)
    matmuls on 32-row PE tile positions. The 4 bands run concurrently
    on the PE's 32-row sub-array strips (~320ns total) - but only when
    issued adjacently, so they go FIRST in each accumulation group
    (start=True) and the feature matmuls close it (stop=True); the
    tile scheduler then has no reason to interleave them,
  * per-row bias SC*(B0 - pos) via a 22nd one-hot row (x valid-col
    indicator, so pad columns stay exactly 0).
- Each half-tile iteration (2048 mem cols) uses TWO psum tiles:
  PA (chunks 0,1 -> ACT) and PB (chunks 2,3 -> DVE). Separate tiles,
  else ACT's in-place exp write and DVE's read of the other region
  serialize on whole-tile dependency tracking (~850ns/iter).
- Consumers:
  * ACT exps PA (scale=1/SC, bias=-B0) with accum_out row sums -> SSA,
  * DVE clamps PB to int16 = clamp(psum, 0, 32512) whose bf16 bitcast
    IS Schraudolph exp; the tile is DMA'd raw to DRAM and the HOST
    does the exp-decode + row sum (a device reduce would bottleneck
    DVE; the Pool engine cannot read PSUM and has ~700ns/instr
    overhead, so it cannot help).
- Rows with sim-pos > ~89.5 saturate to huge-finite/inf -> the host's
  -log(1/(S+1+eps)+eps) clamps them to the reference's 18.42 value
  (dead rows), exactly matching the fp32 reference semantics.
"""
from contextlib import ExitStack

import numpy as np
import ml_dtypes

TEMP = 0.1
POS_THRESH = 0.7
EPS = 1e-8
N, C, M, NLAB = 16384, 256, 4000, 21
MP = 4096                  # memory columns padded
NCORES = 8
RPC = N // NCORES          # 2048 rows per core
NT = RPC // 128            # 16 n-tiles per core
HALF = 2048                # mem cols per half-tile iteration
J = 512                    # matmul chunk width (psum bank)
XA = 1024                  # cols 0:XA -> ACT exp; XA:2048 -> DVE clamp
YB = HALF - XA             # DVE-path cols (1024), exported raw as int16

SC = 128.0 / np.log(2.0)            # 184.6635
SHIFT = 0.0579                      # schraudolph centering (in ln2 units)
B0 = (127.0 - SHIFT) * np.log(2.0)  # 87.9896  (psum bias; ACT cancels it)
SQ = float(np.sqrt(SC / TEMP))      # 42.9725  (fp8 per-side scale)
CAPV = 32512.0                      # 0x7F00 -> bf16 1.66e38 (huge finite)

_cache = {}


def _build():
    import concourse.bacc as bacc
    import concourse.tile as tile
    from concourse import mybir

    f32 = mybir.dt.float32
    bf16 = mybir.dt.bfloat16
    f8 = mybir.dt.float8e4
    i16 = mybir.dt.int16
    Alu = mybir.AluOpType
    Act = mybir.ActivationFunctionType
    DR = mybir.MatmulPerfMode.DoubleRow

    nc = bacc.Bacc(None)

    # DRAM params. ext: [C, RPC] fp8 (planes = the two 128-row K-halves).
    ext1_d = nc.declare_dram_parameter("ext1", [C, RPC], f8, isOutput=False)
    ext2_d = nc.declare_dram_parameter("ext2", [C, RPC], f8, isOutput=False)
    mem_d = nc.declare_dram_parameter("extmem", [C, MP], f8, isOutput=False)
    eqa1_d = nc.declare_dram_parameter("eqanc1", [128, RPC], bf16, isOutput=False)
    eqa2_d = nc.declare_dram_parameter("eqanc2", [128, RPC], bf16, isOutput=False)
    eqm_d = nc.declare_dram_parameter("eqmem", [128, MP], f8, isOutput=False)
    out_d = nc.declare_dram_parameter("out", [128, 4 * NT], f32, isOutput=True)
    # DVE-path clamped int16 psum (bf16 bitcast = schraudolph exp); the
    # host does the exp-decode + row sum.
    exd_d = nc.declare_dram_parameter(
        "exd", [128, 2 * NT * 2 * YB], i16, isOutput=True
    )

    T0C = 4  # tiles in the startup DMA chunk

    with tile.TileContext(nc) as tc, ExitStack() as ctx:
        consts = ctx.enter_context(tc.tile_pool(name="consts", bufs=1))
        psum = ctx.enter_context(tc.tile_pool(name="psum", bufs=2, space="PSUM"))
        sb = ctx.enter_context(tc.tile_pool(name="sb", bufs=3))

        # ---- resident inputs, ordered by first use ----
        e1 = consts.tile([128, 2, RPC], f8, tag="e1", name="e1")
        e2 = consts.tile([128, 2, RPC], f8, tag="e2", name="e2")
        mem = consts.tile([128, 2, MP], f8, tag="mem", name="mem")
        eqa1 = consts.tile([128, RPC], bf16, tag="eqa1", name="eqa1")
        eqa2 = consts.tile([128, RPC], bf16, tag="eqa2", name="eqa2")
        eqm = consts.tile([128, MP], f8, tag="eqm", name="eqm")

        # startup chunk covers the ENTIRE first half-tile iteration
        # (mem cols 0:2048 of both K-planes + eqm) so the PE does not
        # starve mid-iteration while the bulk remainder streams in.
        t0c = T0C * 128
        h0 = slice(0, HALF)
        rest = slice(HALF, MP)
        nc.sync.dma_start(out=mem[:, 0, h0], in_=mem_d[0:128, h0])
        nc.sync.dma_start(out=mem[:, 1, h0], in_=mem_d[128:256, h0])
        nc.sync.dma_start(out=eqm[:, h0], in_=eqm_d[:, h0])
        nc.sync.dma_start(out=eqa1[:, 0:t0c], in_=eqa1_d[:, 0:t0c])
        nc.sync.dma_start(out=e1[:, 0, 0:t0c], in_=ext1_d[0:128, 0:t0c])
        nc.sync.dma_start(out=e1[:, 1, 0:t0c], in_=ext1_d[128:256, 0:t0c])
        # the h0 round visits tiles 4.. before any h1 work, so ext/eqa
        # remainders come before the second memory half.
        nc.sync.dma_start(out=e1[:, 0, t0c:], in_=ext1_d[0:128, t0c:])
        nc.sync.dma_start(out=e1[:, 1, t0c:], in_=ext1_d[128:256, t0c:])
        nc.sync.dma_start(out=eqa1[:, t0c:], in_=eqa1_d[:, t0c:])
        nc.sync.dma_start(out=mem[:, 0, rest], in_=mem_d[0:128, rest])
        nc.sync.dma_start(out=mem[:, 1, rest], in_=mem_d[128:256, rest])
        nc.sync.dma_start(out=eqm[:, rest], in_=eqm_d[:, rest])
        nc.sync.dma_start(out=e2[:, 0, :], in_=ext2_d[0:128, :])
        nc.sync.dma_start(out=e2[:, 1, :], in_=ext2_d[128:256, :])
        nc.sync.dma_start(out=eqa2[:], in_=eqa2_d[:])

        biasA = consts.tile([128, 1], f32, tag="biasA", name="biasA")
        nc.vector.memset(biasA[:], -B0)

        # per-half-tile ACT row sums.
        SSA = consts.tile([128, 2, NT, 2], f32, tag="SSA", name="SSA")

        # h-major: the whole h0 round (all 16 tiles x mem cols 0:2048)
        # runs on just the first half of the memory bank, giving the
        # bulk DMA ~40us of cover before any h1 column is touched.
        for b, (ekt, eqa) in enumerate([(e1, eqa1), (e2, eqa2)]):
            for h in range(2):
                for t in range(NT):
                    tc0 = t * 128
                    base = h * HALF
                    PA = psum.tile([128, XA], f32, tag="PA", name=f"PA{b}_{t}_{h}")
                    PB = psum.tile([128, YB], f32, tag="PB", name=f"PB{b}_{t}_{h}")
                    lhsT = ekt[:, :, tc0 : tc0 + 128]
                    # mask matmuls first (see module docstring). Feature
                    # order (0,1,2,3): PA closes ~580ns earlier, so the
                    # ACT chain (the longest: 1088 exp + 208 accum-read)
                    # starts earlier and its completion no longer stalls
                    # the masks of iteration i+2 on the reused buffer.
                    for ci in (0, 1, 2, 3):
                        c0 = base + ci * J
                        pt, o0 = (PB, (ci - 2) * J) if ci >= 2 else (PA, ci * J)
                        nc.tensor.matmul(
                            pt[:, o0 : o0 + J],
                            eqa[32 * ci : 32 * ci + NLAB + 1, tc0 : tc0 + 128],
                            eqm[32 * ci : 32 * ci + NLAB + 1, c0 : c0 + J],
                            start=True,
                            stop=False,
                            tile_position=(32 * ci, 0),
                        )
                    for ci in (0, 1, 2, 3):
                        c0 = base + ci * J
                        pt, o0 = (PB, (ci - 2) * J) if ci >= 2 else (PA, ci * J)
                        nc.tensor.matmul(
                            pt[:, o0 : o0 + J],
                            lhsT,
                            mem[:, :, c0 : c0 + J],
                            start=False,
                            stop=True,
                            perf_mode=DR,
                        )

                    # PA -> ACT: exp((psum/SC) - B0), accum row sum.
                    # Emitted before the DVE clamp: ACT is the critical
                    # chain, so it gets the earlier scheduler priority.
                    nc.scalar.activation(
                        out=PA[:],
                        in_=PA[:],
                        func=Act.Exp,
                        bias=biasA[:],
                        scale=float(1.0 / SC),
                        accum_out=SSA[:, b, t, h : h + 1],
                    )

                    # PB -> DVE: int16 = clamp(psum, 0, 32512);
                    # bf16 bitcast IS schraudolph exp, decoded host-side.
                    ex = sb.tile([128, YB], i16, tag="ex", name=f"ex{b}_{t}_{h}")
                    nc.vector.tensor_scalar(
                        out=ex[:],
                        in0=PB[:],
                        scalar1=0.0,
                        scalar2=CAPV,
                        op0=Alu.max,
                        op1=Alu.min,
                    )
                    e0 = ((b * NT + t) * 2 + h) * YB
                    nc.sync.dma_start(
                        out=exd_d[:, e0 : e0 + YB], in_=ex[:]
                    )

        nc.sync.dma_start(
            out=out_d[:, 0 : 4 * NT], in_=SSA[:].rearrange("p b t h -> p (b t h)")
        )

    nc.finalize()
    return nc


def _host_prep(inputs):
    bf = ml_dtypes.bfloat16
    f8 = ml_dtypes.float8_e4m3
    f1 = np.ascontiguousarray(np.asarray(inputs["output_feat1"], np.float32))
    f2 = np.ascontiguousarray(np.asarray(inputs["output_feat2"], np.float32))
    l1 = np.asarray(inputs["pseudo_label1"], np.int32)
    l2 = np.asarray(inputs["pseudo_label2"], np.int32)
    ul1 = np.asarray(inputs["output_ul1"], np.float32)
    ul2 = np.asarray(inputs["output_ul2"], np.float32)
    i1 = np.asarray(inputs["selected_idx1"], np.int64)
    i2 = np.asarray(inputs["selected_idx2"], np.int64)

    b, c, h, w = ul1.shape
    u1 = ul1.transpose(0, 2, 3, 1).reshape(b * h * w, c)
    u2 = ul2.transpose(0, 2, 3, 1).reshape(b * h * w, c)
    mem = np.concatenate([u1[i1], u2[i2]], axis=0)               # [M, C]
    memlab = np.concatenate([l1[i1], l2[i2]], axis=0)            # [M]

    pos = (f1 * f2).sum(axis=1, dtype=np.float64) / TEMP         # [N] exact

    extmem = np.zeros((C, MP), np.float32)
    extmem[:, :M] = mem.T * SQ
    extmem = extmem.astype(f8)                                   # [C, MP]

    # mask memory side: rows 32u+i = onehot(memlab==i); row 32u+21 = valid
    lab_eye = np.arange(NLAB, dtype=np.int32)
    eqmem = np.zeros((128, MP), np.float32)
    oh_mem = (memlab[None, :] == lab_eye[:, None]).astype(np.float32)
    for u in range(4):
        eqmem[32 * u : 32 * u + NLAB, :M] = oh_mem
        eqmem[32 * u + NLAB, :M] = 1.0
    eqmem = eqmem.astype(f8)

    def eq_anchor(lab, pos_sl):
        out = np.zeros((128, lab.shape[0]), np.float32)
        oh = (-SC * 1000.0) * (lab[None, :] == lab_eye[:, None])
        brow = SC * (B0 - pos_sl)
        for u in range(4):
            out[32 * u : 32 * u + NLAB] = oh
            out[32 * u + NLAB] = brow
        return out.astype(bf)

    def pack_ext(x):   # [RPC, C] fp32 -> [C, RPC] fp8e4m3
        return np.ascontiguousarray((x * SQ).T).astype(f8)

    in_maps = []
    for cix in range(NCORES):
        sl = slice(cix * RPC, (cix + 1) * RPC)
        in_maps.append({
            "ext1": pack_ext(f1[sl]),
            "ext2": pack_ext(f2[sl]),
            "extmem": extmem,
            "eqanc1": np.ascontiguousarray(eq_anchor(l1[sl], pos[sl])),
            "eqanc2": np.ascontiguousarray(eq_anchor(l2[sl], pos[sl])),
            "eqmem": eqmem,
        })
    return in_maps, pos


def _finalize(results, inputs):
    g1 = np.asarray(inputs["pseudo_logits1"], np.float64)
    g2 = np.asarray(inputs["pseudo_logits2"], np.float64)

    # device partials -> S per row, ordered [core, tile, lane]
    S = np.zeros((2, N), np.float64)
    for cix, r in enumerate(results):
        o = np.asarray(r["out"], np.float64)
        st = o[:, 0 : 4 * NT].reshape(128, 2, NT, 2).sum(axis=3)
        # int16 -> bf16 bitcast is the schraudolph exp decode
        ex = (
            np.asarray(r["exd"], np.int16)
            .view(ml_dtypes.bfloat16)
            .astype(np.float64)
        )
        st += ex.reshape(128, 2, NT, 2 * YB).sum(axis=3)
        for b in range(2):
            # row (cix*RPC + t*128 + lane) <- st[lane, b, t]
            S[b, cix * RPC : (cix + 1) * RPC] = st[:, b].T.reshape(RPC)

    S = np.nan_to_num(S, nan=np.inf, posinf=np.inf, neginf=0.0)
    with np.errstate(divide="ignore", over="ignore"):
        sig = 1.0 / (S + 1.0 + EPS)
        lam = -np.log(sig + EPS)                     # per-row loss term

    m1 = ((g2 > POS_THRESH) & (g1 < g2)).astype(np.float64)
    m2 = ((g1 > POS_THRESH) & (g2 < g1)).astype(np.float64)
    loss = (lam[0] * m1).sum() / (m1.sum() + 1e-12) + \
           (lam[1] * m2).sum() / (m2.sum() + 1e-12)
    return np.float32(loss)


def _run(inputs, trace=False):
    from concourse.bass_utils import run_bass_kernel_spmd

    if "nc" not in _cache:
        _cache["nc"] = _build()
    in_maps, _pos = _host_prep(inputs)
    res = run_bass_kernel_spmd(
        _cache["nc"], in_maps, list(range(NCORES)), trace=trace
    )
    return _finalize(res.results, inputs), res


def kernel(**inputs):
    out, _ = _run(inputs)
    return out


def kernel_with_profile(**inputs):
    out, res = _run(inputs, trace=True)
    return out, res


# revision 34
# speedup vs baseline: 1.1376x; 1.1376x over previous
"""DirectionalContrastiveLoss on 8 TRN2 NeuronCores (Bass/Tile), v7.

Data-parallel over the N=16384 anchor rows (2048 rows/core); the 4000-row
memory bank is replicated (padded to 4096 columns with zero features).

Device algorithm (per core):
- PSUM holds SC*(sim - pos + B0), SC = 184.664 = 2^7/ln2, B0 = 87.99:
  * features as fp8e4m3 DoubleRow matmuls: K=256 contraction in ONE pass
    via the [128, 2, cols] plane AP (planes = the two 128-row K-halves);
    512-col chunks stream at 2 fp8 elem/cycle (~216ns/chunk),
  * label mask as bf16 -SC*1000*onehot(label) x onehot(mem_label)
    matmuls on 32-row PE tile positions. The 4 bands run concurrently
    on the PE's 32-row sub-array strips (~320ns total) - but only when
    issued adjacently, so they go FIRST in each accumulation group
    (start=True) and the feature matmuls close it (stop=True); the
    tile scheduler then has no reason to interleave them,
  * per-row bias SC*(B0 - pos) via a 22nd one-hot row (x valid-col
    indicator, so pad columns stay exactly 0).
- Each half-tile iteration (2048 mem cols) uses TWO psum tiles:
  PA (chunks 0,1 -> ACT) and PB (chunks 2,3 -> DVE). Separate tiles,
  else ACT's in-place exp write and DVE's read of the other region
  serialize on whole-tile dependency tracking (~850ns/iter).
- Consumers:
  * ACT exps PA (scale=1/SC, bias=-B0) with accum_out row sums -> SSA,
  * DVE clamps PB to int16 = clamp(psum, 0, 32512) whose bf16 bitcast
    IS Schraudolph exp; the tile is DMA'd raw to DRAM and the HOST
    does the exp-decode + row sum (a device reduce would bottleneck
    DVE; the Pool engine cannot read PSUM and has ~700ns/instr
    overhead, so it cannot help).
- Rows with sim-pos > ~89.5 saturate to huge-finite/inf -> the host's
  -log(1/(S+1+eps)+eps) clamps them to the reference's 18.42 value
  (dead rows), exactly matching the fp32 reference semantics.
"""
from contextlib import ExitStack

import numpy as np
import ml_dtypes

TEMP = 0.1
POS_THRESH = 0.7
EPS = 1e-8
N, C, M, NLAB = 16384, 256, 4000, 21
MP = 4096                  # memory columns padded
NCORES = 8
RPC = N // NCORES          # 2048 rows per core
NT = RPC // 128            # 16 n-tiles per core
HALF = 2048                # mem cols per half-tile iteration
J = 512                    # matmul chunk width (psum bank)
XA = 1024                  # cols 0:XA -> ACT exp; XA:2048 -> DVE clamp
YB = HALF - XA             # DVE-path cols (1024), exported raw as int16

SC = 128.0 / np.log(2.0)            # 184.6635
SHIFT = 0.0579                      # schraudolph centering (in ln2 units)
B0 = (127.0 - SHIFT) * np.log(2.0)  # 87.9896  (psum bias; ACT cancels it)
SQ = float(np.sqrt(SC / TEMP))      # 42.9725  (fp8 per-side scale)
CAPV = 32512.0                      # 0x7F00 -> bf16 1.66e38 (huge finite)

_cache = {}


def _build():
    import concourse.bacc as bacc
    import concourse.tile as tile
    from concourse import mybir

    f32 = mybir.dt.float32
    bf16 = mybir.dt.bfloat16
    f8 = mybir.dt.float8e4
    i16 = mybir.dt.int16
    Alu = mybir.AluOpType
    Act = mybir.ActivationFunctionType
    DR = mybir.MatmulPerfMode.DoubleRow

    nc = bacc.Bacc(None)

    # DRAM params. ext: [C, RPC] fp8 (planes = the two 128-row K-halves).
    ext1_d = nc.declare_dram_parameter("ext1", [C, RPC], f8, isOutput=False)
    ext2_d = nc.declare_dram_parameter("ext2", [C, RPC], f8, isOutput=False)
    mem_d = nc.declare_dram_parameter("extmem", [C, MP], f8, isOutput=False)
    eqa1_d = nc.declare_dram_parameter("eqanc1", [128, RPC], bf16, isOutput=False)
    eqa2_d = nc.declare_dram_parameter("eqanc2", [128, RPC], bf16, isOutput=False)
    eqm_d = nc.declare_dram_parameter("eqmem", [128, MP], f8, isOutput=False)
    out_d = nc.declare_dram_parameter("out", [128, 4 * NT], f32, isOutput=True)
    # DVE-path clamped int16 psum (bf16 bitcast = schraudolph exp); the
    # host does the exp-decode + row sum.
    exd_d = nc.declare_dram_parameter(
        "exd", [128, 2 * NT * 2 * YB], i16, isOutput=True
    )

    T0C = 4  # tiles in the startup DMA chunk

    with tile.TileContext(nc) as tc, ExitStack() as ctx:
        consts = ctx.enter_context(tc.tile_pool(name="consts", bufs=1))
        psum = ctx.enter_context(tc.tile_pool(name="psum", bufs=2, space="PSUM"))
        # Deep pool for the export tiles: early in the run the ~4.6MB of
        # input streaming owns the DMA queues, so per-iteration exports
        # backlog; 16 bufs let the DVE clamp run ~16 iterations ahead of
        # the export drain instead of stalling after 3.
        sb = ctx.enter_context(tc.tile_pool(name="sb", bufs=16))

        # ---- resident inputs, ordered by first use ----
        e1 = consts.tile([128, 2, RPC], f8, tag="e1", name="e1")
        e2 = consts.tile([128, 2, RPC], f8, tag="e2", name="e2")
        mem = consts.tile([128, 2, MP], f8, tag="mem", name="mem")
        eqa1 = consts.tile([128, RPC], bf16, tag="eqa1", name="eqa1")
        eqa2 = consts.tile([128, RPC], bf16, tag="eqa2", name="eqa2")
        eqm = consts.tile([128, MP], f8, tag="eqm", name="eqm")

        # startup chunk covers the ENTIRE first half-tile iteration
        # (mem cols 0:2048 of both K-planes + eqm) so the PE does not
        # starve mid-iteration while the bulk remainder streams in.
        t0c = T0C * 128
        h0 = slice(0, HALF)
        rest = slice(HALF, MP)
        nc.sync.dma_start(out=mem[:, 0, h0], in_=mem_d[0:128, h0])
        nc.sync.dma_start(out=mem[:, 1, h0], in_=mem_d[128:256, h0])
        nc.sync.dma_start(out=eqm[:, h0], in_=eqm_d[:, h0])
        nc.sync.dma_start(out=eqa1[:, 0:t0c], in_=eqa1_d[:, 0:t0c])
        nc.sync.dma_start(out=e1[:, 0, 0:t0c], in_=ext1_d[0:128, 0:t0c])
        nc.sync.dma_start(out=e1[:, 1, 0:t0c], in_=ext1_d[128:256, 0:t0c])
        # the h0 round visits tiles 4.. before any h1 work, so ext/eqa
        # remainders come before the second memory half.
        nc.sync.dma_start(out=e1[:, 0, t0c:], in_=ext1_d[0:128, t0c:])
        nc.sync.dma_start(out=e1[:, 1, t0c:], in_=ext1_d[128:256, t0c:])
        nc.sync.dma_start(out=eqa1[:, t0c:], in_=eqa1_d[:, t0c:])
        nc.sync.dma_start(out=mem[:, 0, rest], in_=mem_d[0:128, rest])
        nc.sync.dma_start(out=mem[:, 1, rest], in_=mem_d[128:256, rest])
        nc.sync.dma_start(out=eqm[:, rest], in_=eqm_d[:, rest])
        nc.sync.dma_start(out=e2[:, 0, :], in_=ext2_d[0:128, :])
        nc.sync.dma_start(out=e2[:, 1, :], in_=ext2_d[128:256, :])
        nc.sync.dma_start(out=eqa2[:], in_=eqa2_d[:])

        biasA = consts.tile([128, 1], f32, tag="biasA", name="biasA")
        nc.vector.memset(biasA[:], -B0)

        # per-half-tile ACT row sums.
        SSA = consts.tile([128, 2, NT, 2], f32, tag="SSA", name="SSA")

        # h-major: the whole h0 round (all 16 tiles x mem cols 0:2048)
        # runs on just the first half of the memory bank, giving the
        # bulk DMA ~40us of cover before any h1 column is touched.
        for b, (ekt, eqa) in enumerate([(e1, eqa1), (e2, eqa2)]):
            for h in range(2):
                for t in range(NT):
                    tc0 = t * 128
                    base = h * HALF
                    PA = psum.tile([128, XA], f32, tag="PA", name=f"PA{b}_{t}_{h}")
                    PB = psum.tile([128, YB], f32, tag="PB", name=f"PB{b}_{t}_{h}")
                    lhsT = ekt[:, :, tc0 : tc0 + 128]
                    # mask matmuls first (see module docstring). Feature
                    # order (0,1,2,3): PA closes ~580ns earlier, so the
                    # ACT chain (the longest: 1088 exp + 208 accum-read)
                    # starts earlier and its completion no longer stalls
                    # the masks of iteration i+2 on the reused buffer.
                    for ci in (0, 1, 2, 3):
                        c0 = base + ci * J
                        pt, o0 = (PB, (ci - 2) * J) if ci >= 2 else (PA, ci * J)
                        nc.tensor.matmul(
                            pt[:, o0 : o0 + J],
                            eqa[32 * ci : 32 * ci + NLAB + 1, tc0 : tc0 + 128],
                            eqm[32 * ci : 32 * ci + NLAB + 1, c0 : c0 + J],
                            start=True,
                            stop=False,
                            tile_position=(32 * ci, 0),
                        )
                    for ci in (0, 1, 2, 3):
                        c0 = base + ci * J
                        pt, o0 = (PB, (ci - 2) * J) if ci >= 2 else (PA, ci * J)
                        nc.tensor.matmul(
                            pt[:, o0 : o0 + J],
                            lhsT,
                            mem[:, :, c0 : c0 + J],
                            start=False,
                            stop=True,
                            perf_mode=DR,
                        )

                    # PA -> ACT: exp((psum/SC) - B0), accum row sum.
                    # Emitted before the DVE clamp: ACT is the critical
                    # chain, so it gets the earlier scheduler priority.
                    nc.scalar.activation(
                        out=PA[:],
                        in_=PA[:],
                        func=Act.Exp,
                        bias=biasA[:],
                        scale=float(1.0 / SC),
                        accum_out=SSA[:, b, t, h : h + 1],
                    )

                    # PB -> DVE: int16 = clamp(psum, 0, 32512);
                    # bf16 bitcast IS schraudolph exp, decoded host-side.
                    ex = sb.tile([128, YB], i16, tag="ex", name=f"ex{b}_{t}_{h}")
                    nc.vector.tensor_scalar(
                        out=ex[:],
                        in0=PB[:],
                        scalar1=0.0,
                        scalar2=CAPV,
                        op0=Alu.max,
                        op1=Alu.min,
                    )
                    e0 = ((b * NT + t) * 2 + h) * YB
                    nc.sync.dma_start(
                        out=exd_d[:, e0 : e0 + YB], in_=ex[:]
                    )

        nc.sync.dma_start(
            out=out_d[:, 0 : 4 * NT], in_=SSA[:].rearrange("p b t h -> p (b t h)")
        )

    nc.finalize()
    return nc


def _host_prep(inputs):
    bf = ml_dtypes.bfloat16
    f8 = ml_dtypes.float8_e4m3
    f1 = np.ascontiguousarray(np.asarray(inputs["output_feat1"], np.float32))
    f2 = np.ascontiguousarray(np.asarray(inputs["output_feat2"], np.float32))
    l1 = np.asarray(inputs["pseudo_label1"], np.int32)
    l2 = np.asarray(inputs["pseudo_label2"], np.int32)
    ul1 = np.asarray(inputs["output_ul1"], np.float32)
    ul2 = np.asarray(inputs["output_ul2"], np.float32)
    i1 = np.asarray(inputs["selected_idx1"], np.int64)
    i2 = np.asarray(inputs["selected_idx2"], np.int64)

    b, c, h, w = ul1.shape
    u1 = ul1.transpose(0, 2, 3, 1).reshape(b * h * w, c)
    u2 = ul2.transpose(0, 2, 3, 1).reshape(b * h * w, c)
    mem = np.concatenate([u1[i1], u2[i2]], axis=0)               # [M, C]
    memlab = np.concatenate([l1[i1], l2[i2]], axis=0)            # [M]

    pos = (f1 * f2).sum(axis=1, dtype=np.float64) / TEMP         # [N] exact

    extmem = np.zeros((C, MP), np.float32)
    extmem[:, :M] = mem.T * SQ
    extmem = extmem.astype(f8)                                   # [C, MP]

    # mask memory side: rows 32u+i = onehot(memlab==i); row 32u+21 = valid
    lab_eye = np.arange(NLAB, dtype=np.int32)
    eqmem = np.zeros((128, MP), np.float32)
    oh_mem = (memlab[None, :] == lab_eye[:, None]).astype(np.float32)
    for u in range(4):
        eqmem[32 * u : 32 * u + NLAB, :M] = oh_mem
        eqmem[32 * u + NLAB, :M] = 1.0
    eqmem = eqmem.astype(f8)

    def eq_anchor(lab, pos_sl):
        out = np.zeros((128, lab.shape[0]), np.float32)
        oh = (-SC * 1000.0) * (lab[None, :] == lab_eye[:, None])
        brow = SC * (B0 - pos_sl)
        for u in range(4):
            out[32 * u : 32 * u + NLAB] = oh
            out[32 * u + NLAB] = brow
        return out.astype(bf)

    def pack_ext(x):   # [RPC, C] fp32 -> [C, RPC] fp8e4m3
        return np.ascontiguousarray((x * SQ).T).astype(f8)

    in_maps = []
    for cix in range(NCORES):
        sl = slice(cix * RPC, (cix + 1) * RPC)
        in_maps.append({
            "ext1": pack_ext(f1[sl]),
            "ext2": pack_ext(f2[sl]),
            "extmem": extmem,
            "eqanc1": np.ascontiguousarray(eq_anchor(l1[sl], pos[sl])),
            "eqanc2": np.ascontiguousarray(eq_anchor(l2[sl], pos[sl])),
            "eqmem": eqmem,
        })
    return in_maps, pos


def _finalize(results, inputs):
    g1 = np.asarray(inputs["pseudo_logits1"], np.float64)
    g2 = np.asarray(inputs["pseudo_logits2"], np.float64)

    # device partials -> S per row, ordered [core, tile, lane]
    S = np.zeros((2, N), np.float64)
    for cix, r in enumerate(results):
        o = np.asarray(r["out"], np.float64)
        st = o[:, 0 : 4 * NT].reshape(128, 2, NT, 2).sum(axis=3)
        # int16 -> bf16 bitcast is the schraudolph exp decode
        ex = (
            np.asarray(r["exd"], np.int16)
            .view(ml_dtypes.bfloat16)
            .astype(np.float64)
        )
        st += ex.reshape(128, 2, NT, 2 * YB).sum(axis=3)
        for b in range(2):
            # row (cix*RPC + t*128 + lane) <- st[lane, b, t]
            S[b, cix * RPC : (cix + 1) * RPC] = st[:, b].T.reshape(RPC)

    S = np.nan_to_num(S, nan=np.inf, posinf=np.inf, neginf=0.0)
    with np.errstate(divide="ignore", over="ignore"):
        sig = 1.0 / (S + 1.0 + EPS)
        lam = -np.log(sig + EPS)                     # per-row loss term

    m1 = ((g2 > POS_THRESH) & (g1 < g2)).astype(np.float64)
    m2 = ((g1 > POS_THRESH) & (g2 < g1)).astype(np.float64)
    loss = (lam[0] * m1).sum() / (m1.sum() + 1e-12) + \
           (lam[1] * m2).sum() / (m2.sum() + 1e-12)
    return np.float32(loss)


def _run(inputs, trace=False):
    from concourse.bass_utils import run_bass_kernel_spmd

    if "nc" not in _cache:
        _cache["nc"] = _build()
    in_maps, _pos = _host_prep(inputs)
    res = run_bass_kernel_spmd(
        _cache["nc"], in_maps, list(range(NCORES)), trace=trace
    )
    return _finalize(res.results, inputs), res


def kernel(**inputs):
    out, _ = _run(inputs)
    return out


def kernel_with_profile(**inputs):
    out, res = _run(inputs, trace=True)
    return out, res


# revision 38
# speedup vs baseline: 1.2512x; 1.0999x over previous
"""DirectionalContrastiveLoss on 8 TRN2 NeuronCores (Bass/Tile), v11.

Data-parallel over the N=16384 anchor rows (2048 rows/core); the 4000-row
memory bank is replicated (padded to 4096 columns with zero features).

Device algorithm (per core):
- PSUM holds SC*(sim - pos + B0), SC = 184.664 = 2^7/ln2, B0 = 87.99:
  * features as fp8e4m3 DoubleRow matmuls: K=256 contraction in ONE pass
    via the [128, 2, cols] plane AP (planes = the two 128-row K-halves);
    512-col chunks stream at 2 fp8 elem/cycle (~216ns/chunk),
  * label mask as bf16 -SC*1000*onehot(label) x onehot(mem_label)
    matmuls on 32-row PE tile positions. The 4 bands run concurrently
    on the PE's 32-row sub-array strips (~320ns total) - but only when
    issued adjacently, so they go FIRST in each accumulation group
    (start=True) and the feature matmuls close it (stop=True); the
    tile scheduler then has no reason to interleave them,
  * per-row bias SC*(B0 - pos) via a 22nd one-hot row (x valid-col
    indicator, so pad columns stay exactly 0).
- Each half-tile iteration (2048 mem cols) uses TWO psum tiles:
  PA (chunks 0,1 -> ACT) and PB (chunks 2,3 -> DVE). Separate tiles,
  else ACT's in-place exp write and DVE's read of the other region
  serialize on whole-tile dependency tracking (~850ns/iter).
- Consumers:
  * ACT exps PA (scale=1/SC, bias=-B0) with accum_out row sums -> SSA,
  * DVE clamps PB to int16 = clamp(psum, 0, 32512) whose bf16 bitcast
    IS Schraudolph exp; the tile is DMA'd raw to DRAM and the HOST
    does the exp-decode + row sum (a device reduce would bottleneck
    DVE; the Pool engine cannot read PSUM and has ~700ns/instr
    overhead, so it cannot help).
- Rows with sim-pos > ~89.5 saturate to huge-finite/inf -> the host's
  -log(1/(S+1+eps)+eps) clamps them to the reference's 18.42 value
  (dead rows), exactly matching the fp32 reference semantics.
"""
from contextlib import ExitStack

import numpy as np
import ml_dtypes

TEMP = 0.1
POS_THRESH = 0.7
EPS = 1e-8
N, C, M, NLAB = 16384, 256, 4000, 21
MP = 4096                  # memory columns padded
NCORES = 8
RPC = N // NCORES          # 2048 rows per core
NT = RPC // 128            # 16 n-tiles per core
HALF = 2048                # mem cols per half-tile iteration
J = 512                    # matmul chunk width (psum bank)
XA = 1024                  # cols 0:XA -> ACT exp; XA:2048 -> DVE clamp
YB = HALF - XA             # DVE-path cols (1024), exported raw as int16

SC = 128.0 / np.log(2.0)            # 184.6635
SHIFT = 0.0579                      # schraudolph centering (in ln2 units)
B0 = (127.0 - SHIFT) * np.log(2.0)  # 87.9896  (psum bias; ACT cancels it)
SQ = float(np.sqrt(SC / TEMP))      # 42.9725  (fp8 per-side scale)
CAPV = 32512.0                      # 0x7F00 -> bf16 1.66e38 (huge finite)

_cache = {}


def _build():
    import concourse.bacc as bacc
    import concourse.tile as tile
    from concourse import mybir

    f32 = mybir.dt.float32
    bf16 = mybir.dt.bfloat16
    f8 = mybir.dt.float8e4
    i16 = mybir.dt.int16
    Alu = mybir.AluOpType
    Act = mybir.ActivationFunctionType
    DR = mybir.MatmulPerfMode.DoubleRow

    nc = bacc.Bacc(None)

    # DRAM params. ext: [C, RPC] fp8 (planes = the two 128-row K-halves).
    ext1_d = nc.declare_dram_parameter("ext1", [C, RPC], f8, isOutput=False)
    ext2_d = nc.declare_dram_parameter("ext2", [C, RPC], f8, isOutput=False)
    mem_d = nc.declare_dram_parameter("extmem", [C, MP], f8, isOutput=False)
    eqa1_d = nc.declare_dram_parameter("eqanc1", [128, RPC], bf16, isOutput=False)
    eqa2_d = nc.declare_dram_parameter("eqanc2", [128, RPC], bf16, isOutput=False)
    eqm_d = nc.declare_dram_parameter("eqmem", [128, MP], f8, isOutput=False)
    out_d = nc.declare_dram_parameter("out", [128, 4 * NT], f32, isOutput=True)
    # DVE-path clamped int16 psum (bf16 bitcast = schraudolph exp); the
    # host does the exp-decode + row sum.
    exd_d = nc.declare_dram_parameter(
        "exd", [128, 2 * NT * 2 * YB], i16, isOutput=True
    )

    T0C = 4  # tiles in the startup DMA chunk

    with tile.TileContext(nc) as tc, ExitStack() as ctx:
        consts = ctx.enter_context(tc.tile_pool(name="consts", bufs=1))
        # Deep pool for the export tiles: early in the run the ~4.6MB of
        # input streaming owns the DMA queues, so per-iteration exports
        # backlog; 16 bufs let the DVE clamp run ~16 iterations ahead of
        # the export drain instead of stalling after 3.
        sb = ctx.enter_context(tc.tile_pool(name="sb", bufs=16))

        # Warmup: the PE idles ~3us at low p-state (0.65GHz) while the
        # input DMAs stream; a few dummy DoubleRow matmuls (garbage
        # operands, result never read) plus one ACT exp and one DVE
        # clamp ramp the clocks during that window for free. The
        # transient PSUM pool closes before the main pool opens, so the
        # main loop still gets all 8 banks.
        wa = consts.tile([128, 2, 128], f8, tag="wa", name="wa")
        wb = consts.tile([128, 2, 512], f8, tag="wb", name="wb")
        wex = sb.tile([128, 512], i16, tag="wex", name="wex")
        nc.vector.memset(wa[:], 0.0)
        nc.vector.memset(wb[:], 0.0)
        with tc.tile_pool(name="warm", bufs=1, space="PSUM") as warm:
            wp = warm.tile([128, 512], f32, tag="wp", name="wp")
            for k in range(6):
                nc.tensor.matmul(
                    wp[:], wa[:], wb[:], start=(k == 0), stop=(k == 5),
                    perf_mode=DR,
                )
            nc.scalar.activation(
                out=wp[:], in_=wp[:], func=Act.Exp, bias=0.0, scale=1.0
            )
            nc.vector.tensor_scalar(
                out=wex[:], in0=wp[:], scalar1=0.0, scalar2=CAPV,
                op0=Alu.max, op1=Alu.min,
            )

        psum = ctx.enter_context(tc.tile_pool(name="psum", bufs=2, space="PSUM"))

        # ---- resident inputs, ordered by first use ----
        e1 = consts.tile([128, 2, RPC], f8, tag="e1", name="e1")
        e2 = consts.tile([128, 2, RPC], f8, tag="e2", name="e2")
        mem = consts.tile([128, 2, MP], f8, tag="mem", name="mem")
        eqa1 = consts.tile([128, RPC], bf16, tag="eqa1", name="eqa1")
        eqa2 = consts.tile([128, RPC], bf16, tag="eqa2", name="eqa2")
        eqm = consts.tile([128, MP], f8, tag="eqm", name="eqm")

        # startup chunk covers the ENTIRE first half-tile iteration
        # (mem cols 0:2048 of both K-planes + eqm) so the PE does not
        # starve mid-iteration while the bulk remainder streams in.
        t0c = T0C * 128
        h0 = slice(0, HALF)
        rest = slice(HALF, MP)
        nc.sync.dma_start(out=mem[:, 0, h0], in_=mem_d[0:128, h0])
        nc.sync.dma_start(out=mem[:, 1, h0], in_=mem_d[128:256, h0])
        nc.sync.dma_start(out=eqm[:, h0], in_=eqm_d[:, h0])
        nc.sync.dma_start(out=eqa1[:, 0:t0c], in_=eqa1_d[:, 0:t0c])
        nc.sync.dma_start(out=e1[:, 0, 0:t0c], in_=ext1_d[0:128, 0:t0c])
        nc.sync.dma_start(out=e1[:, 1, 0:t0c], in_=ext1_d[128:256, 0:t0c])
        # the h0 round visits tiles 4.. before any h1 work, so ext/eqa
        # remainders come before the second memory half.
        nc.sync.dma_start(out=e1[:, 0, t0c:], in_=ext1_d[0:128, t0c:])
        nc.sync.dma_start(out=e1[:, 1, t0c:], in_=ext1_d[128:256, t0c:])
        nc.sync.dma_start(out=eqa1[:, t0c:], in_=eqa1_d[:, t0c:])
        nc.sync.dma_start(out=mem[:, 0, rest], in_=mem_d[0:128, rest])
        nc.sync.dma_start(out=mem[:, 1, rest], in_=mem_d[128:256, rest])
        nc.sync.dma_start(out=eqm[:, rest], in_=eqm_d[:, rest])
        nc.sync.dma_start(out=e2[:, 0, :], in_=ext2_d[0:128, :])
        nc.sync.dma_start(out=e2[:, 1, :], in_=ext2_d[128:256, :])
        nc.sync.dma_start(out=eqa2[:], in_=eqa2_d[:])

        biasA = consts.tile([128, 1], f32, tag="biasA", name="biasA")
        nc.vector.memset(biasA[:], -B0)

        # per-half-tile ACT row sums.
        SSA = consts.tile([128, 2, NT, 2], f32, tag="SSA", name="SSA")

        # h-major: the whole h0 round (all 16 tiles x mem cols 0:2048)
        # runs on just the first half of the memory bank, giving the
        # bulk DMA ~40us of cover before any h1 column is touched.
        for b, (ekt, eqa) in enumerate([(e1, eqa1), (e2, eqa2)]):
            for h in range(2):
                for t in range(NT):
                    tc0 = t * 128
                    base = h * HALF
                    PA = psum.tile([128, XA], f32, tag="PA", name=f"PA{b}_{t}_{h}")
                    PB = psum.tile([128, YB], f32, tag="PB", name=f"PB{b}_{t}_{h}")
                    lhsT = ekt[:, :, tc0 : tc0 + 128]
                    # mask matmuls first (see module docstring). Feature
                    # order (0,1,2,3): PA closes ~580ns earlier, so the
                    # ACT chain (the longest: 1088 exp + 208 accum-read)
                    # starts earlier and its completion no longer stalls
                    # the masks of iteration i+2 on the reused buffer.
                    for ci in (0, 1, 2, 3):
                        c0 = base + ci * J
                        pt, o0 = (PB, (ci - 2) * J) if ci >= 2 else (PA, ci * J)
                        nc.tensor.matmul(
                            pt[:, o0 : o0 + J],
                            eqa[32 * ci : 32 * ci + NLAB + 1, tc0 : tc0 + 128],
                            eqm[32 * ci : 32 * ci + NLAB + 1, c0 : c0 + J],
                            start=True,
                            stop=False,
                            tile_position=(32 * ci, 0),
                        )
                    for ci in (0, 1, 2, 3):
                        c0 = base + ci * J
                        pt, o0 = (PB, (ci - 2) * J) if ci >= 2 else (PA, ci * J)
                        nc.tensor.matmul(
                            pt[:, o0 : o0 + J],
                            lhsT,
                            mem[:, :, c0 : c0 + J],
                            start=False,
                            stop=True,
                            perf_mode=DR,
                        )

                    # PA -> ACT: exp((psum/SC) - B0), accum row sum.
                    # Emitted before the DVE clamp: ACT is the critical
                    # chain, so it gets the earlier scheduler priority.
                    nc.scalar.activation(
                        out=PA[:],
                        in_=PA[:],
                        func=Act.Exp,
                        bias=biasA[:],
                        scale=float(1.0 / SC),
                        accum_out=SSA[:, b, t, h : h + 1],
                    )

                    # PB -> DVE: int16 = clamp(psum, 0, 32512);
                    # bf16 bitcast IS schraudolph exp, decoded host-side.
                    ex = sb.tile([128, YB], i16, tag="ex", name=f"ex{b}_{t}_{h}")
                    nc.vector.tensor_scalar(
                        out=ex[:],
                        in0=PB[:],
                        scalar1=0.0,
                        scalar2=CAPV,
                        op0=Alu.max,
                        op1=Alu.min,
                    )
                    e0 = ((b * NT + t) * 2 + h) * YB
                    nc.sync.dma_start(
                        out=exd_d[:, e0 : e0 + YB], in_=ex[:]
                    )

        nc.sync.dma_start(
            out=out_d[:, 0 : 4 * NT], in_=SSA[:].rearrange("p b t h -> p (b t h)")
        )

    nc.finalize()
    return nc


def _host_prep(inputs):
    bf = ml_dtypes.bfloat16
    f8 = ml_dtypes.float8_e4m3
    f1 = np.ascontiguousarray(np.asarray(inputs["output_feat1"], np.float32))
    f2 = np.ascontiguousarray(np.asarray(inputs["output_feat2"], np.float32))
    l1 = np.asarray(inputs["pseudo_label1"], np.int32)
    l2 = np.asarray(inputs["pseudo_label2"], np.int32)
    ul1 = np.asarray(inputs["output_ul1"], np.float32)
    ul2 = np.asarray(inputs["output_ul2"], np.float32)
    i1 = np.asarray(inputs["selected_idx1"], np.int64)
    i2 = np.asarray(inputs["selected_idx2"], np.int64)

    b, c, h, w = ul1.shape
    u1 = ul1.transpose(0, 2, 3, 1).reshape(b * h * w, c)
    u2 = ul2.transpose(0, 2, 3, 1).reshape(b * h * w, c)
    mem = np.concatenate([u1[i1], u2[i2]], axis=0)               # [M, C]
    memlab = np.concatenate([l1[i1], l2[i2]], axis=0)            # [M]

    pos = (f1 * f2).sum(axis=1, dtype=np.float64) / TEMP         # [N] exact

    extmem = np.zeros((C, MP), np.float32)
    extmem[:, :M] = mem.T * SQ
    extmem = extmem.astype(f8)                                   # [C, MP]

    # mask memory side: rows 32u+i = onehot(memlab==i); row 32u+21 = valid
    lab_eye = np.arange(NLAB, dtype=np.int32)
    eqmem = np.zeros((128, MP), np.float32)
    oh_mem = (memlab[None, :] == lab_eye[:, None]).astype(np.float32)
    for u in range(4):
        eqmem[32 * u : 32 * u + NLAB, :M] = oh_mem
        eqmem[32 * u + NLAB, :M] = 1.0
    eqmem = eqmem.astype(f8)

    def eq_anchor(lab, pos_sl):
        out = np.zeros((128, lab.shape[0]), np.float32)
        oh = (-SC * 1000.0) * (lab[None, :] == lab_eye[:, None])
        brow = SC * (B0 - pos_sl)
        for u in range(4):
            out[32 * u : 32 * u + NLAB] = oh
            out[32 * u + NLAB] = brow
        return out.astype(bf)

    def pack_ext(x):   # [RPC, C] fp32 -> [C, RPC] fp8e4m3
        return np.ascontiguousarray((x * SQ).T).astype(f8)

    in_maps = []
    for cix in range(NCORES):
        sl = slice(cix * RPC, (cix + 1) * RPC)
        in_maps.append({
            "ext1": pack_ext(f1[sl]),
            "ext2": pack_ext(f2[sl]),
            "extmem": extmem,
            "eqanc1": np.ascontiguousarray(eq_anchor(l1[sl], pos[sl])),
            "eqanc2": np.ascontiguousarray(eq_anchor(l2[sl], pos[sl])),
            "eqmem": eqmem,
        })
    return in_maps, pos


def _finalize(results, inputs):
    g1 = np.asarray(inputs["pseudo_logits1"], np.float64)
    g2 = np.asarray(inputs["pseudo_logits2"], np.float64)

    # device partials -> S per row, ordered [core, tile, lane]
    S = np.zeros((2, N), np.float64)
    for cix, r in enumerate(results):
        o = np.asarray(r["out"], np.float64)
        st = o[:, 0 : 4 * NT].reshape(128, 2, NT, 2).sum(axis=3)
        # int16 -> bf16 bitcast is the schraudolph exp decode
        ex = (
            np.asarray(r["exd"], np.int16)
            .view(ml_dtypes.bfloat16)
            .astype(np.float64)
        )
        st += ex.reshape(128, 2, NT, 2 * YB).sum(axis=3)
        for b in range(2):
            # row (cix*RPC + t*128 + lane) <- st[lane, b, t]
            S[b, cix * RPC : (cix + 1) * RPC] = st[:, b].T.reshape(RPC)

    S = np.nan_to_num(S, nan=np.inf, posinf=np.inf, neginf=0.0)
    with np.errstate(divide="ignore", over="ignore"):
        sig = 1.0 / (S + 1.0 + EPS)
        lam = -np.log(sig + EPS)                     # per-row loss term

    m1 = ((g2 > POS_THRESH) & (g1 < g2)).astype(np.float64)
    m2 = ((g1 > POS_THRESH) & (g2 < g1)).astype(np.float64)
    loss = (lam[0] * m1).sum() / (m1.sum() + 1e-12) + \
           (lam[1] * m2).sum() / (m2.sum() + 1e-12)
    return np.float32(loss)


def _run(inputs, trace=False):
    from concourse.bass_utils import run_bass_kernel_spmd

    if "nc" not in _cache:
        _cache["nc"] = _build()
    in_maps, _pos = _host_prep(inputs)
    res = run_bass_kernel_spmd(
        _cache["nc"], in_maps, list(range(NCORES)), trace=trace
    )
    return _finalize(res.results, inputs), res


def kernel(**inputs):
    out, _ = _run(inputs)
    return out


def kernel_with_profile(**inputs):
    out, res = _run(inputs, trace=True)
    return out, res
